# revision 33
# baseline (speedup 1.0000x reference)
"""Trainium2 Bass kernel for nn_AttentionBlock (sparse causal attention).

Math (per batch b, head h), A = r_prime[b] (T x N):
    out[b] = sum_h tril(A Q_h A^T) @ (A E_h^T)
Data-parallel over batch (8 batches -> 8 NeuronCores); per core a chunked
(C=128) linear-attention decomposition in bf16 with fp32 PSUM.

Key facts validated on this runtime (quirk_test.py) that differ from the
previous session's assumptions:
- matmul operands MAY live at SBUF partition base 64 (lhsT and rhs must
  share the base), so the odd-head half of C = (A Q)^T is read in place
  against a copy of rpt duplicated on partitions 64..127 (the old c_hi
  relocation DMA is gone).
- matmul MAY write PSUM at partition offset 64, so the running state is
  accumulated parity-stacked ([j of even heads; j of odd heads] x
  (pair, i)), which halves the snapshot copy (one [128,256] DVE copy)
  and halves the inter MM count (4 pair-stacked K=128 MMs per chunk).
- matmul PSUM output must be fp32 (bf16 PSUM rejected by bass).

Per chunk: W = A_I C for all 8 heads into one [128,1024] fp32 PSUM pair
(double-buffered, prefetched one chunk ahead), wm = W*tril-mask as a
single DVE tensor_mul, out[t,i] += sum_h wm_h^T er_h (8 N=64 MMs, FWL
weight loads) + sum_pair C_pair^T S_pair (4 MMs); state P += A_I^T Er_I
(2 parity MMs). Engine balance: DVE does mask+state snapshot, ACT does
C/Er/out evacuations. Output is accumulated [t, i]-major in one PSUM
bank per 8 chunks, so the host does a cheap reshape (no transpose).

This session's HW findings (36.5us -> ~27.5us):
- The machine is engine-queue-order bound, not engine-busy bound: DVE/ACT
  sequencers are strict FIFO, so a small state-snapshot copy emitted at the
  TOP of the chunk head-of-line blocks the mask/evac copies queued behind
  it. Emitting the snapshot AFTER the w/er/c emissions (s16late=1) and
  keeping it on DVE (s16alt=0) cut ~9us. With that fixed, deeper prefetch
  (cpre=3) and deeper SBUF pools (wmb/erb/s16b = 6/6/4) pay off.
- Structural rewrites all regress on HW despite better cost-model numbers:
  g=64 sub-chunk packing (halves the DVE mask FD but doubles small-op
  count) measured 59us; merging er+c2 into one ACT copy (cmerge) 40us;
  splitting the mask (mdve2/mps/mact) +4..7us. Op COUNT dominates op SIZE.
- GPSIMD (Pool) cannot access PSUM (walrus birverifier). DMA cannot read
  PSUM (bass assert). So PSUM evacuation is strictly DVE+ACT.
- Two concurrent row-tiled matmuls (different lhsT partition bases = PE
  row groups) writing the SAME PSUM bank wedge the device; the baseline's
  e-parity W pair is safe only because e=0/e=1 target different banks.
"""

import numpy as np

import concourse.bacc as bacc
import concourse.bass as bass
import concourse.mybir as mybir
import concourse.tile as tile
from concourse.bass_utils import run_bass_kernel_spmd

D, T, N, H = 8, 2048, 64, 8
C = 128
BF = mybir.dt.bfloat16
F8 = mybir.dt.float8e4
F32 = mybir.dt.float32
NP_BF = mybir.dt.np(BF)


def build_nc(t_len: int = T, reps: int = 1, loop_reps: int = 1,
             unroll: int = 0, s16alt: int = 0, stream: int = 0,
             erfirst: int = 1, hints: int = 0, cpre: int = 3,
             stearly: int = 1,
             wm8: int = 1, obf: int = 0,
             s16p: int = 0, c2p: int = 0, erp: int = 0,
             mps: int = 0, isplit: int = 0, mdve2: int = 0,
             odma: int = 0, mact: int = 0,
             wmb: int = 6, erb: int = 6, s16b: int = 4,
             cb: int = 4, ospan: int = 8, cmerge: int = 0,
             s16late: int = 1, csplit: int = 0, outd: int = 0) -> bacc.Bacc:
    nch = t_len // C
    assert nch % 2 == 0
    if unroll == 0:
        unroll = (8 if loop_reps % 8 == 0 else
                  4 if loop_reps % 4 == 0 else 1)
    nc = bacc.Bacc("TRN2", target_bir_lowering=False, debug=False)

    rp = nc.dram_tensor("rp", [C, nch * N], BF, kind="ExternalInput")
    rpt = nc.dram_tensor("rpt", [N, t_len], BF, kind="ExternalInput")
    q_all = nc.dram_tensor("q_all", [N, H * N], BF, kind="ExternalInput")
    et_all = nc.dram_tensor("et_all", [N, H * N], BF, kind="ExternalInput")
    maskd = nc.dram_tensor("maskd", [C, 8 * C], BF, kind="ExternalInput")
    out_t = nc.dram_tensor("out_t", [C, nch * N],
                           mybir.dt.bfloat16 if obf else F32,
                           kind="ExternalOutput")

    with tile.TileContext(nc) as tc:
        with (
            tc.tile_pool(name="const", bufs=1) as cpool,
            tc.tile_pool(name="csb", bufs=cb) as c_pool,
            tc.tile_pool(name="ersb", bufs=erb) as er_pool,
            tc.tile_pool(name="wm", bufs=wmb) as wm_pool,
            tc.tile_pool(name="s16p", bufs=s16b) as s16_pool,
            tc.tile_pool(name="ps_c", bufs=1, space="PSUM") as ps_c,
            tc.tile_pool(name="ps_w", bufs=2, space="PSUM") as ps_w,
            tc.tile_pool(name="ps_er", bufs=1, space="PSUM") as ps_er,
            tc.tile_pool(name="ps_s", bufs=1, space="PSUM") as ps_s,
            tc.tile_pool(name="ps_o", bufs=(1 if ospan == 8 else 2),
                         space="PSUM") as ps_o,
        ):
            q_sb = cpool.tile([N, H * N], BF)
            nc.gpsimd.dma_start(q_sb[:], q_all[:])
            et_sb = cpool.tile([N, H * N], BF)
            nc.gpsimd.dma_start(et_sb[:], et_all[:])
            # rpt duplicated on both partition halves: base-64 operands for
            # the odd-head W matmuls
            rpt_sb = cpool.tile([2 * N, t_len], BF)
            for pc in range(4):
                sl = slice(pc * t_len // 4, (pc + 1) * t_len // 4)
                nc.sync.dma_start(rpt_sb[0:N, sl], rpt[:, sl])
                nc.sync.dma_start(rpt_sb[N : 2 * N, sl], rpt[:, sl])
            rp_sb = cpool.tile([C, nch * N], BF)
            for pc in range(2):
                sl = slice(pc * nch * N // 2, (pc + 1) * nch * N // 2)
                nc.scalar.dma_start(rp_sb[:, sl], rp[:, sl])
            mask_sb = cpool.tile([C, 8 * C], BF)
            nc.gpsimd.dma_start(mask_sb[:], maskd[:])
            out_sb = cpool.tile([C, nch * N], BF if obf else F32)
            scr = cpool.tile([1, 4], BF)
            nc.vector.tensor_copy(scr[:], mask_sb[:1, :4])

            def make_stream():
                state = {"p_s": None, "p_o2": None}
                c_tiles = {}
                er_tiles = {}
                wm_tiles = {}

                def emit_c(u, ii, qsel=None):
                    # C for both chunks of pair ii, all 8 heads:
                    # c2_sb[64e+k, 256p+128m+t] = C_{2p+e}[k, t(of chunk m)]
                    # cmerge layout instead: [c_q0 | er_A | c_q1 | er_B] with
                    # c_q at q*1024, er slots at 512/1536
                    psl = slice(ii * 2 * C, (ii + 1) * 2 * C)
                    if cmerge:
                        c2_sb = c_pool.tile([2 * N, 16 * C], BF, tag="c2")
                        for q in range(2):
                            p_c = ps_c.tile([2 * N, 8 * C], F32, tag="c")
                            for pp in range(2):
                                p = 2 * q + pp
                                nc.tensor.matmul(
                                    p_c[:, pp * 2 * C : (pp + 1) * 2 * C],
                                    lhsT=q_sb[:, p * 2 * N : (p + 1) * 2 * N],
                                    rhs=rpt_sb[0:N, psl],
                                    start=(pp == 0),
                                    stop=(pp == 1),
                                )
                            nc.scalar.copy(
                                c2_sb[:, q * 8 * C : q * 8 * C + 4 * C],
                                p_c[:, 0 : 4 * C],
                            )
                        c_tiles[(u, ii)] = c2_sb
                        return
                    if qsel in (None, 0):
                        c2_sb = c_pool.tile([2 * N, 8 * C], BF, tag="c2")
                        c_tiles[(u, ii)] = c2_sb
                    else:
                        c2_sb = c_tiles[(u, ii)]
                    for q in ((0, 1) if qsel is None else (qsel,)):
                        p_c = ps_c.tile([2 * N, 4 * C], F32, tag="c")
                        for pp in range(2):
                            p = 2 * q + pp
                            nc.tensor.matmul(
                                p_c[:, pp * 2 * C : (pp + 1) * 2 * C],
                                lhsT=q_sb[:, p * 2 * N : (p + 1) * 2 * N],
                                rhs=rpt_sb[0:N, psl],
                                start=(pp == 0),
                                stop=(pp == 1),
                            )
                        if c2p == 1 or (c2p == 2 and q == 1):
                            nc.gpsimd.tensor_copy(
                                c2_sb[:, q * 4 * C : (q + 1) * 4 * C], p_c[:]
                            )
                        else:
                            nc.scalar.copy(
                                c2_sb[:, q * 4 * C : (q + 1) * 4 * C], p_c[:]
                            )

                def emit_ce(u, i):
                    # merged emission at chunk i: c-half (q=i%2) of pair
                    # jj=i//2+cpre, plus er(i+1), evacuated by ONE ACT copy
                    # into the pair's c3 tile at cols q*1024 : q*1024+1024
                    ii, m = i // 2, i % 2
                    jj = ii + cpre
                    have_c = jj < nch // 2
                    have_er = i + 1 < nch
                    if not (have_c or have_er):
                        return
                    p_cer = ps_c.tile([2 * N, 8 * C], F32, tag="c")
                    if have_c:
                        psl = slice(jj * 2 * C, (jj + 1) * 2 * C)
                        if m == 0:
                            c3_sb = c_pool.tile([2 * N, 16 * C], BF,
                                                tag="c2", name="c3_sb")
                            c_tiles[(u, jj)] = c3_sb
                        c3 = c_tiles[(u, jj)]
                        for pp in range(2):
                            p = 2 * m + pp
                            nc.tensor.matmul(
                                p_cer[:, pp * 2 * C : (pp + 1) * 2 * C],
                                lhsT=q_sb[:, p * 2 * N : (p + 1) * 2 * N],
                                rhs=rpt_sb[0:N, psl],
                                start=(pp == 0),
                                stop=(pp == 1),
                            )
                    if have_er:
                        tsl = slice((i + 1) * C, (i + 2) * C)
                        nc.tensor.matmul(
                            p_cer[:, 4 * C : 8 * C],
                            lhsT=rpt_sb[0:N, tsl], rhs=et_sb[:],
                            start=True, stop=True,
                        )
                    if have_c and have_er:
                        nc.scalar.copy(
                            c3[:, m * 8 * C : (m + 1) * 8 * C], p_cer[:]
                        )
                        er_tiles[(u, i + 1)] = (c3, m * 8 * C + 4 * C)
                    elif have_c:
                        nc.scalar.copy(
                            c3[:, m * 8 * C : m * 8 * C + 4 * C],
                            p_cer[:, 0 : 4 * C],
                        )
                    else:
                        er_sb = er_pool.tile([C, H * N], BF, tag="er_sb")
                        nc.scalar.copy(er_sb[:], p_cer[:, 4 * C : 8 * C])
                        er_tiles[(u, i + 1)] = (er_sb, 0)

                def emit_er(u, i):
                    tsl = slice(i * C, (i + 1) * C)
                    if cmerge:
                        p_er = ps_c.tile([2 * N, 8 * C], F32, tag="c")
                        nc.tensor.matmul(
                            p_er[:, 4 * C : 8 * C],
                            lhsT=rpt_sb[0:N, tsl], rhs=et_sb[:],
                            start=True, stop=True,
                        )
                        er_sb = er_pool.tile([C, H * N], BF, tag="er_sb")
                        nc.scalar.copy(er_sb[:], p_er[:, 4 * C : 8 * C])
                        er_tiles[(u, i)] = (er_sb, 0)
                        return
                    p_er = ps_er.tile([C, H * N], F32, tag="er")
                    nc.tensor.matmul(
                        p_er[:], lhsT=rpt_sb[0:N, tsl], rhs=et_sb[:],
                        start=True, stop=True,
                    )
                    er_sb = er_pool.tile([C, H * N], BF, tag="er_sb")
                    if erp:
                        nc.gpsimd.tensor_copy(er_sb[:], p_er[:])
                    else:
                        nc.scalar.copy(er_sb[:], p_er[:])
                    er_tiles[(u, i)] = er_sb

                def emit_w(u, i):
                    # W+mask for chunk i, all heads; odd heads read C's hi
                    # half via base-64 operands
                    ii, m = i // 2, i % 2
                    c2_sb = c_tiles[(u, ii)]
                    tsl = slice(i * C, (i + 1) * C)
                    p_w = ps_w.tile([C, 8 * C], F32, tag="w")
                    for e in range(2):
                        csrc = c2_sb[0:N, :] if e == 0 else c2_sb[N : 2 * N, :]
                        if cmerge:
                            c_v = csrc.rearrange(
                                "k (q z pp mm t) -> k q z pp mm t",
                                q=2, z=2, pp=2, mm=2,
                            )
                            rhs = c_v[:, :, 0, :, m, :]
                        else:
                            c_v = csrc.rearrange(
                                "k (p mm t) -> k p mm t", p=4, mm=2
                            )
                            rhs = c_v[:, :, m, :]
                        nc.tensor.matmul(
                            p_w[:, e * 4 * C : (e + 1) * 4 * C],
                            lhsT=rpt_sb[e * N : (e + 1) * N, tsl],
                            rhs=rhs,
                            start=True,
                            stop=True,
                        )
                    wm = wm_pool.tile([C, 8 * C], F8 if wm8 else BF,
                                      tag="wm")
                    if mps:
                        sp = (8 - mps) * C
                        nc.vector.tensor_mul(wm[:, :sp], p_w[:, :sp],
                                             mask_sb[:, :sp])
                        nc.gpsimd.tensor_mul(wm[:, sp:], p_w[:, sp:],
                                             mask_sb[:, sp:])
                    elif mdve2:
                        nc.vector.tensor_mul(wm[:, : 4 * C], p_w[:, : 4 * C],
                                             mask_sb[:, : 4 * C])
                        nc.vector.tensor_mul(wm[:, 4 * C :], p_w[:, 4 * C :],
                                             mask_sb[:, 4 * C :])
                    elif mact:
                        nc.scalar.copy(wm[:], p_w[:])
                    else:
                        nc.vector.tensor_mul(wm[:], p_w[:], mask_sb[:])
                    wm_tiles[(u, i)] = wm

                def preamble(u):
                    for jj in range(min(cpre, nch // 2)):
                        emit_c(u, jj)
                    emit_er(u, 0)
                    emit_w(u, 0)

                def body(u, last):
                    # one rep's 16 chunks; unless `last`, the next rep's
                    # preamble is emitted during the final chunk
                    for ii in range(nch // 2):
                        c2_sb = c_tiles[(u, ii)]
                        for m in range(2):
                            i = 2 * ii + m
                            chunk(u, ii, m, i, c2_sb)
                            if stream and i == nch - 2 and not last:
                                preamble(u + 1)

                def snap(i):
                    p_s = state["p_s"]
                    s16f = s16_pool.tile(
                        [2 * N, 4 * N], BF, tag="s16f"
                    )
                    if s16p:
                        nc.gpsimd.tensor_copy(s16f[:], p_s[:])
                    elif s16alt == 2:
                        nc.scalar.copy(s16f[:], p_s[:])
                    elif s16alt and i % 2 == 1:
                        nc.scalar.copy(s16f[:], p_s[:])
                    else:
                        nc.vector.tensor_copy(s16f[:], p_s[:])
                    return s16f

                def chunk(u, ii, m, i, c2_sb):
                    if True:
                        # parity-stacked state snapshot (before P update)
                        s16f = None
                        if i > 0 and not s16late:
                            s16f = snap(i)
                        if erfirst == 2 and i + 1 < nch:
                            emit_er(u, i + 1)
                        if i + 1 < nch:
                            emit_w(u, i + 1)
                            if cmerge:
                                emit_ce(u, i)
                            elif erfirst == 1:
                                emit_er(u, i + 1)
                        if not cmerge and ii + cpre < nch // 2:
                            if csplit:
                                emit_c(u, ii + cpre, qsel=m)
                            elif m == 0:
                                emit_c(u, ii + cpre)
                        if not cmerge and i + 1 < nch and not erfirst:
                            emit_er(u, i + 1)
                        if i > 0 and s16late:
                            s16f = snap(i)
                        ert = er_tiles.pop((u, i))
                        if cmerge:
                            er_t, er_b = ert
                            er_sb = er_t[:, er_b : er_b + H * N]
                        else:
                            er_sb = ert[:]
                        wm = wm_tiles.pop((u, i))

                        def emit_state(i, er_sb):
                            # state update, parity-stacked: lo partitions
                            # even heads' P, hi partitions odd heads'
                            if i >= nch - 1:
                                return
                            if i == 0:
                                p_s_new = ps_s.tile([2 * N, 4 * N], F32,
                                                    tag="s")
                                state["p_s"] = p_s_new
                            p_s = state["p_s"]
                            for e in range(2):
                                nc.tensor.matmul(
                                    p_s[e * N : (e + 1) * N, :],
                                    lhsT=rp_sb[:, i * N : (i + 1) * N],
                                    rhs=er_sb[:, e * 4 * N : (e + 1) * 4 * N],
                                    start=(i == 0),
                                    stop=(i == nch - 2),
                                    skip_group_check=True,
                                )

                        if stearly:
                            emit_state(i, er_sb)

                        s = i % ospan
                        if s == 0:
                            p_o2 = ps_o.tile([C, ospan * N], F32, tag="o")
                            state["p_o2"] = p_o2
                        p_o2 = state["p_o2"]
                        p_o = p_o2[:, s * N : (s + 1) * N]
                        n_base = 8 * (1 + isplit)
                        n_mm = n_base if i == 0 else n_base + 4
                        g = 0
                        for h in (0, 2, 4, 6, 1, 3, 5, 7):
                            e, gh = h % 2, h // 2
                            ge = e * 4 + gh
                            if isplit:
                                for q2 in range(2):
                                    nc.tensor.matmul(
                                        p_o[:, q2 * N // 2 : (q2 + 1) * N // 2],
                                        lhsT=wm[:, (e * 4 + gh) * C :
                                                (e * 4 + gh + 1) * C],
                                        rhs=er_sb[:, ge * N + q2 * N // 2 :
                                                  ge * N + (q2 + 1) * N // 2],
                                        start=(g == 0 or g == 1),
                                        stop=(g >= n_mm - 2) if i == 0 else False,
                                        skip_group_check=True,
                                    )
                                    g += 1
                                continue
                            nc.tensor.matmul(
                                p_o,
                                lhsT=wm[:, (e * 4 + gh) * C :
                                        (e * 4 + gh + 1) * C],
                                rhs=er_sb[:, ge * N : (ge + 1) * N],
                                start=(g == 0),
                                stop=(g == n_mm - 1),
                                skip_group_check=True,
                            )
                            g += 1
                        if i > 0:
                            # inter: pair-stacked K=128 against the
                            # parity-stacked snapshot
                            for p in range(4):
                                if cmerge:
                                    cc = ((p // 2) * 8 * C + (p % 2) * 2 * C
                                          + m * C)
                                else:
                                    cc = p * 2 * C + m * C
                                nc.tensor.matmul(
                                    p_o,
                                    lhsT=c2_sb[:, cc : cc + C],
                                    rhs=s16f[:, p * N : (p + 1) * N],
                                    start=False,
                                    stop=(g == n_mm - 1),
                                    skip_group_check=True,
                                )
                                g += 1
                        if s == ospan - 1:
                            w8 = i // ospan
                            osl = slice(w8 * ospan * N, (w8 + 1) * ospan * N)
                            if odma:
                                nc.sync.dma_start(out_t[:, osl], p_o2[:])
                            elif outd:
                                nc.vector.tensor_copy(out_sb[:, osl], p_o2[:])
                                nc.sync.dma_start(out_t[:, osl], out_sb[:, osl])
                            else:
                                nc.scalar.copy(out_sb[:, osl], p_o2[:])
                                nc.sync.dma_start(out_t[:, osl], out_sb[:, osl])

                        if not stearly:
                            emit_state(i, er_sb)

                return preamble, body

            preamble, body = make_stream()

            def run_group(n):
                if stream:
                    preamble(0)
                    for u in range(n):
                        body(u, last=(u == n - 1))
                else:
                    for u in range(n):
                        preamble(u)
                        body(u, last=True)

            if loop_reps > 1:
                assert loop_reps % unroll == 0
                hint_sets = {
                    0: (),
                    1: (mybir.EngineType.PE, mybir.EngineType.Activation,
                        mybir.EngineType.DVE, mybir.EngineType.SP),
                    2: (mybir.EngineType.PE, mybir.EngineType.Activation,
                        mybir.EngineType.DVE, mybir.EngineType.SP,
                        mybir.EngineType.Pool),
                }
                with tc.For_i(
                    0, loop_reps // unroll, 1,
                    hint_engines=hint_sets[hints],
                ):
                    run_group(unroll)
            else:
                run_group(reps)

    nc.compile()
    return nc


def build_nc_g64(t_len: int = T, reps: int = 1, loop_reps: int = 1,
                 unroll: int = 0, cpre: int = 2, s16d: int = 1,
                 odma: int = 0, wm8: int = 1) -> bacc.Bacc:
    """g=64 sub-chunk variant: per 128-token macro-chunk, the two 64-token
    sub-chunks' diagonal W blocks are packed on partition halves so the DVE
    mask op is one [128, 512] TensorTensor (vs [128, 1024]); the (t1, u0)
    rectangular coupling flows through the state at 64-token cadence.
    K=64/M=64 matmuls land on disjoint PE quadrants (tile_position derived
    from partition bases) so they overlap in the array."""
    nch = t_len // C
    assert nch % 2 == 0
    if unroll == 0:
        unroll = (8 if loop_reps % 8 == 0 else
                  4 if loop_reps % 4 == 0 else 1)
    nc = bacc.Bacc("TRN2", target_bir_lowering=False, debug=False)

    rp = nc.dram_tensor("rp", [C, nch * N], BF, kind="ExternalInput")
    rpt = nc.dram_tensor("rpt", [N, t_len], BF, kind="ExternalInput")
    q_all = nc.dram_tensor("q_all", [N, H * N], BF, kind="ExternalInput")
    et_all = nc.dram_tensor("et_all", [N, H * N], BF, kind="ExternalInput")
    maskd = nc.dram_tensor("maskd", [C, 8 * N], BF, kind="ExternalInput")
    out_t = nc.dram_tensor("out_t", [C, nch * N], F32, kind="ExternalOutput")

    with tile.TileContext(nc) as tc:
        with (
            tc.tile_pool(name="const", bufs=1) as cpool,
            tc.tile_pool(name="csb", bufs=4) as c_pool,
            tc.tile_pool(name="ersb", bufs=4) as er_pool,
            tc.tile_pool(name="wm", bufs=4) as wm_pool,
            tc.tile_pool(name="s16p", bufs=4) as s16_pool,
            tc.tile_pool(name="ps_c", bufs=1, space="PSUM") as ps_c,
            tc.tile_pool(name="ps_w", bufs=2, space="PSUM") as ps_w,
            tc.tile_pool(name="ps_er", bufs=1, space="PSUM") as ps_er,
            tc.tile_pool(name="ps_s", bufs=1, space="PSUM") as ps_s,
            tc.tile_pool(name="ps_o", bufs=(1 if ospan == 8 else 2),
                         space="PSUM") as ps_o,
        ):
            q_sb = cpool.tile([N, H * N], BF)
            nc.gpsimd.dma_start(q_sb[:], q_all[:])
            et_sb = cpool.tile([N, H * N], BF)
            nc.gpsimd.dma_start(et_sb[:], et_all[:])
            rpt_sb = cpool.tile([2 * N, t_len], BF)
            for pc in range(4):
                sl = slice(pc * t_len // 4, (pc + 1) * t_len // 4)
                nc.sync.dma_start(rpt_sb[0:N, sl], rpt[:, sl])
                nc.sync.dma_start(rpt_sb[N : 2 * N, sl], rpt[:, sl])
            rp_sb = cpool.tile([C, nch * N], BF)
            for pc in range(2):
                sl = slice(pc * nch * N // 2, (pc + 1) * nch * N // 2)
                nc.scalar.dma_start(rp_sb[:, sl], rp[:, sl])
            mask_sb = cpool.tile([C, 8 * N], BF)
            nc.gpsimd.dma_start(mask_sb[:], maskd[:])
            out_sb = cpool.tile([C, nch * N], F32)
            scr = cpool.tile([1, 4], BF)
            nc.vector.tensor_copy(scr[:], mask_sb[:1, :4])

            def make_stream():
                state = {"p_s": None, "p_o2": None}
                c_tiles = {}
                er_tiles = {}
                wm_tiles = {}

                def emit_c(u, ii):
                    psl = slice(ii * 2 * C, (ii + 1) * 2 * C)
                    if qsel in (None, 0):
                        c2_sb = c_pool.tile([2 * N, 8 * C], BF, tag="c2")
                        c_tiles[(u, ii)] = c2_sb
                    else:
                        c2_sb = c_tiles[(u, ii)]
                    for q in ((0, 1) if qsel is None else (qsel,)):
                        p_c = ps_c.tile([2 * N, 4 * C], F32, tag="c")
                        for pp in range(2):
                            p = 2 * q + pp
                            nc.tensor.matmul(
                                p_c[:, pp * 2 * C : (pp + 1) * 2 * C],
                                lhsT=q_sb[:, p * 2 * N : (p + 1) * 2 * N],
                                rhs=rpt_sb[0:N, psl],
                                start=(pp == 0),
                                stop=(pp == 1),
                            )
                        nc.scalar.copy(
                            c2_sb[:, q * 4 * C : (q + 1) * 4 * C], p_c[:]
                        )
                    c_tiles[(u, ii)] = c2_sb

                def emit_er(u, i):
                    tsl = slice(i * C, (i + 1) * C)
                    p_er = ps_er.tile([C, H * N], F32, tag="er")
                    nc.tensor.matmul(
                        p_er[:], lhsT=rpt_sb[0:N, tsl], rhs=et_sb[:],
                        start=True, stop=True,
                    )
                    er_sb = er_pool.tile([C, H * N], BF, tag="er_sb")
                    nc.scalar.copy(er_sb[:], p_er[:])
                    er_tiles[(u, i)] = er_sb

                def emit_w(u, i):
                    # diagonal 64-blocks only, both subs packed on partition
                    # halves. PSUM bank = e (= PE row group of the MM): two
                    # concurrent row-tiled MMs must not share a PSUM bank
                    # (same-bank pairs wedge the device), so p_w is padded to
                    # two banks with the e*256-col data at e*512.
                    ii, m = i // 2, i % 2
                    c2_sb = c_tiles[(u, ii)]
                    p_w = ps_w.tile([C, 16 * N], F32, tag="w")
                    for e in range(2):
                        csrc = c2_sb[0:N, :] if e == 0 else c2_sb[N : 2 * N, :]
                        c_v = csrc.rearrange(
                            "k (p mm t) -> k p mm t", p=4, mm=2
                        )
                        for s in range(2):
                            nc.tensor.matmul(
                                p_w[s * N : (s + 1) * N,
                                    e * 8 * N : e * 8 * N + 4 * N],
                                lhsT=rpt_sb[e * N : (e + 1) * N,
                                            i * C + s * N : i * C + (s + 1) * N],
                                rhs=c_v[:, :, m, s * N : (s + 1) * N],
                                start=True,
                                stop=True,
                            )
                    wm = wm_pool.tile([C, 8 * N], F8 if wm8 else BF, tag="wm")
                    p_wv = p_w[:].rearrange("t (e pad) -> t e pad", e=2)
                    nc.vector.tensor_mul(
                        wm[:].rearrange("t (e c) -> t e c", e=2),
                        p_wv[:, :, 0 : 4 * N],
                        mask_sb[:].rearrange("t (e c) -> t e c", e=2),
                    )
                    wm_tiles[(u, i)] = wm

                def preamble(u):
                    for jj in range(min(cpre, nch // 2)):
                        emit_c(u, jj)
                    emit_er(u, 0)
                    emit_w(u, 0)

                def body(u, last):
                    for ii in range(nch // 2):
                        for m in range(2):
                            i = 2 * ii + m
                            chunk(u, ii, m, i)

                def snapshot(u, i, s, k):
                    # P through sub (i, s-1) / (i-1, 1); k = parity selector
                    p_s = state["p_s"]
                    s16f = s16_pool.tile([2 * N, 4 * N], BF, tag="s16f")
                    if s16d and k % 2 == 0:
                        nc.vector.tensor_copy(s16f[:], p_s[:])
                    else:
                        nc.scalar.copy(s16f[:], p_s[:])
                    return s16f

                def emit_state(u, i, s, er_sb):
                    # P[x, :] += sum_{t in sub} A[t, x] er[t, :]
                    if i == nch - 1 and s == 1:
                        return
                    if i == 0 and s == 0:
                        p_s_new = ps_s.tile([2 * N, 4 * N], F32, tag="s")
                        state["p_s"] = p_s_new
                    p_s = state["p_s"]
                    for e in range(2):
                        nc.tensor.matmul(
                            p_s[e * N : (e + 1) * N, :],
                            lhsT=rp_sb[s * N : (s + 1) * N,
                                       i * N : (i + 1) * N],
                            rhs=er_sb[s * N : (s + 1) * N,
                                      e * 4 * N : (e + 1) * 4 * N],
                            start=(i == 0 and s == 0),
                            stop=(i == nch - 1 and s == 0),
                            skip_group_check=True,
                        )

                def chunk(u, ii, m, i):
                    if i + 1 < nch:
                        emit_w(u, i + 1)
                        emit_er(u, i + 1)
                    if m == 0 and ii + cpre < nch // 2:
                        emit_c(u, ii + cpre)
                    er_sb = er_tiles.pop((u, i))
                    wm = wm_tiles.pop((u, i))
                    c2_sb = c_tiles[(u, ii)]

                    sc = i % 8
                    if sc == 0:
                        p_o2 = ps_o.tile([C, 8 * N], F32, tag="o")
                        state["p_o2"] = p_o2
                    p_o2 = state["p_o2"]
                    p_o = p_o2[:, sc * N : (sc + 1) * N]

                    for s in range(2):
                        first = (i == 0 and s == 0)
                        s16f = None if first else snapshot(u, i, s, 2 * i + s)
                        emit_state(u, i, s, er_sb)
                        # intra: out[u_s, :] += sum_h wm_s_h^T er_s_h
                        n_mm = 8 if first else 12
                        g = 0
                        for ge in range(8):
                            nc.tensor.matmul(
                                p_o[s * N : (s + 1) * N, :],
                                lhsT=wm[s * N : (s + 1) * N,
                                        ge * N : (ge + 1) * N],
                                rhs=er_sb[s * N : (s + 1) * N,
                                          ge * N : (ge + 1) * N],
                                start=(g == 0),
                                stop=(g == n_mm - 1),
                                skip_group_check=True,
                            )
                            g += 1
                        if not first:
                            # inter vs snapshot: pair-stacked K=128
                            for p in range(4):
                                nc.tensor.matmul(
                                    p_o[s * N : (s + 1) * N, :],
                                    lhsT=c2_sb[:, p * 2 * C + m * C + s * N :
                                               p * 2 * C + m * C + (s + 1) * N],
                                    rhs=s16f[:, p * N : (p + 1) * N],
                                    start=False,
                                    stop=(g == n_mm - 1),
                                    skip_group_check=True,
                                )
                                g += 1
                    if sc == 7:
                        w8 = i // 8
                        osl = slice(w8 * 8 * N, (w8 + 1) * 8 * N)
                        if odma:
                            nc.sync.dma_start(out_t[:, osl], p_o2[:])
                        else:
                            nc.scalar.copy(out_sb[:, osl], p_o2[:])
                            nc.sync.dma_start(out_t[:, osl], out_sb[:, osl])

                return preamble, body

            preamble, body = make_stream()

            def run_group(n):
                for u in range(n):
                    preamble(u)
                    body(u, last=True)

            if loop_reps > 1:
                assert loop_reps % unroll == 0
                with tc.For_i(0, loop_reps // unroll, 1):
                    run_group(unroll)
            else:
                run_group(reps)

    nc.compile()
    return nc


def _host_prep(r_prime: np.ndarray, Q: np.ndarray, E: np.ndarray, t_len: int = T):
    nch = t_len // C
    q_all = np.ascontiguousarray(
        Q.transpose(1, 0, 2).reshape(N, H * N)
    ).astype(NP_BF)
    perm = [0, 2, 4, 6, 1, 3, 5, 7]
    et_all = np.ascontiguousarray(
        E[perm].transpose(2, 0, 1).reshape(N, H * N)
    ).astype(NP_BF)
    mask = np.tile(np.triu(np.ones((C, C), np.float32)), (1, 8)).astype(NP_BF)
    in_maps = []
    for b in range(D):
        a = r_prime[b]
        rp16 = (
            a.reshape(nch, C, N).transpose(1, 0, 2).reshape(C, nch * N)
        ).astype(NP_BF)
        rpt16 = np.ascontiguousarray(a.T).astype(NP_BF)
        in_maps.append(
            {
                "rp": rp16,
                "rpt": rpt16,
                "q_all": q_all,
                "et_all": et_all,
                "maskd": mask,
            }
        )
    return in_maps


def _unshard(res, t_len: int = T):
    nch = t_len // C
    outs = []
    for b in range(D):
        o = np.asarray(res[b]["out_t"], np.float32)
        outs.append(
            o.reshape(C, nch, N).transpose(1, 0, 2).reshape(t_len, N)
        )
    return np.stack(outs).astype(np.float32)


_NC_CACHE: dict = {}


def kernel(r_prime: np.ndarray, Q: np.ndarray, E: np.ndarray) -> np.ndarray:
    r_prime = np.asarray(r_prime, np.float32)
    Q = np.asarray(Q, np.float32)
    E = np.asarray(E, np.float32)
    t_len = r_prime.shape[1]
    if ("nc", t_len) not in _NC_CACHE:
        _NC_CACHE[("nc", t_len)] = build_nc(t_len)
    nc = _NC_CACHE[("nc", t_len)]
    in_maps = _host_prep(r_prime, Q, E, t_len)
    res = run_bass_kernel_spmd(nc, in_maps, list(range(D)))
    return _unshard([res.results[b] for b in range(D)], t_len)

